# revision 6
# baseline (speedup 1.0000x reference)
"""Multi-head self-attention + projector, Trainium2 Bass kernel, 8 NeuronCores.

Reference computation (per batch b):
    Q = X @ Wq + bq; K = X @ Wk + bk; V = X @ Wv + bv      (X: [S, D])
    per head h: P_h = softmax(Q_h K_h^T / sqrt(dh)); A_h = P_h V_h
    Y = concat_h(A_h) @ Wo + bo

Sharding: core i handles batch i//2; the PAIR (2b, 2b+1) splits the
HEADS (tensor parallel): each core projects Q/K/V for its 4 heads only,
over the full sequence, runs attention for those heads over all 2048
queries, then the pair exchanges attended^T halves (0.5 MB AllGather)
so each core can compute Y for its 1024 output rows with all 8 heads.
The host rolls odd cores' X columns by -1024 and permutes Wq/Wk/Wv
columns + Wo rows so one SPMD program serves all 8 cores; with the
roll, "my output queries" are local columns 0:1024 on every core and
the partner's needed half is local columns 1024:2048 on every core.

Algebraic simplifications (all exact w.r.t. softmax):
  - bk dropped: per-query constant in scores, softmax cancels it.
  - bv folded into the output bias on host (softmax rows sum to 1).
  - no max-subtraction in softmax: scores are O(1) for these inputs.

Device pipeline per core:
  phase A: Q^T[256,2048] (+bq), K^T[256,2048], V[2048,4,65] (bf16 with
           a per-head ones column for free softmax row sums). Only
           K-chunk0/Q-chunk0 for the first query block run up front;
           everything else streams into the attention iterations.
  phase B: 8 iterations (qb in 4 blocks of 512 queries x hp in 2 local
           head pairs): stream keys in 128-chunks: scoresT via PE (head
           pair row-tiled 0:64/64:128, concurrent), exp on ACT (bf16),
           attended^T accumulation on PE. Normalize by the row-sum row
           (reciprocal in place + stride-0 broadcast DMA); odd head
           shifts to partitions 64:128 via a small SBUF->SBUF DMA.
  exchange: after the 5th iteration the remote query half of att^T is
           AllGather'd within the pair (gpsimd queue only); the partner
           block is selected with parity-conditional DMAs.
  phase C: Y[q,768] = [attT_local; attT_remote].T @ Wo + bo' per
           128-row q-tile, interleaved into the last iteration + tail.
"""

import numpy as np

import concourse.bass as bass
import concourse.mybir as mybir
import concourse.tile as tile
from concourse import bacc, bass_utils

F32 = mybir.dt.float32
BF16 = mybir.dt.bfloat16

B, S, D, HID, HEADS, DH, VD = 4, 2048, 768, 512, 8, 64, 768
N_CORES = 8
SQ = S // 2  # output query rows per core
HL = HID // 2  # local hidden width (4 heads)
HC_L = HL // 128  # 2 local hidden chunks
DC = D // 128  # 6 contraction chunks for the projections
HC = HID // 128  # 4 hidden chunks for phase C
KT = S // 128  # 16 key chunks
QB = S // 512  # 4 query blocks of 512
HPL = 2  # local head pairs


def _bcast_rows(row_ap, n):
    # [1, F] AP -> [1, n, F] AP: free-dim stride-0 repeat of one SBUF row
    ap = [list(row_ap.ap[0])] + [[0, n]] + [list(d) for d in row_ap.ap[1:]]
    return bass.AP(row_ap.tensor, row_ap.offset, ap)


def _kernel_body(tc):
    nc = tc.nc
    xt_d = nc.dram_tensor("xt", [D, S], BF16, kind="ExternalInput").ap()
    wq_d = nc.dram_tensor("wq", [D, HL], BF16, kind="ExternalInput").ap()
    wk_d = nc.dram_tensor("wk", [D, HL], BF16, kind="ExternalInput").ap()
    wv_d = nc.dram_tensor("wv", [D, HL], BF16, kind="ExternalInput").ap()
    bq_d = nc.dram_tensor("bq", [HL], F32, kind="ExternalInput").ap()
    wo_d = nc.dram_tensor("wo", [HID, VD], BF16, kind="ExternalInput").ap()
    bo_d = nc.dram_tensor("bo2", [VD], F32, kind="ExternalInput").ap()
    y_d = nc.dram_tensor("y", [SQ, VD], F32, kind="ExternalOutput").ap()

    with (
        tc.tile_pool(name="persist", bufs=1) as persist,
        tc.tile_pool(name="mm_ps", bufs=3, space="PSUM") as mm_ps_pool,
        tc.tile_pool(name="att_ps", bufs=1, space="PSUM") as att_ps_pool,
        tc.tile_pool(name="pa_sbuf", bufs=1) as pa_sbuf,
        tc.tile_pool(name="e_pool", bufs=12) as e_pool,
        tc.tile_pool(name="rb_pool", bufs=4) as rb_pool,
        tc.tile_pool(name="tmp_pool", bufs=4) as tmp_pool,
        tc.tile_pool(name="y_sb", bufs=2) as y_sb_pool,
        tc.tile_pool(name="dram", bufs=1, space="DRAM") as dram_pool,
    ):
        # ---- persistent SBUF tensors ----
        wo_sb = persist.tile([128, HC, VD], BF16)
        bo_sb = persist.tile([128, VD], F32)
        bq_sb = persist.tile([128, HC_L], F32)
        qt_sb = persist.tile([128, HC_L, S], BF16)
        kt_sb = persist.tile([128, HC_L, S], BF16)
        # V in [seq, head, 65]: per local head [V(64) | ones]
        v_sb = persist.tile([128, KT, 4, DH + 1], BF16)
        att_sb = persist.tile([128, HC_L, S], BF16)
        att_rsb = persist.tile([128, HC_L, SQ], BF16)
        zero_sb = persist.tile([128, 1], F32)

        nc.vector.memset(zero_sb[:], 0.0)
        nc.vector.memset(v_sb[:, :, :, DH : DH + 1], 1.0)
        nc.sync.dma_start(out=bq_sb[:], in_=bq_d.rearrange("(c p) -> p c", c=HC_L))

        xt_sb = pa_sbuf.tile([128, DC, S], BF16)
        wq_sb = pa_sbuf.tile([128, DC, HL], BF16)
        wk_sb = pa_sbuf.tile([128, DC, HL], BF16)
        wv_sb = pa_sbuf.tile([128, DC, HL], BF16)

        send_b = dram_pool.tile([128, HC_L * SQ], BF16)
        gath_b = dram_pool.tile([2 * 128, HC_L * SQ], BF16)

        # input DMAs: weight chunk c + the first query-block column of X
        # first (so the upfront K/Q jobs can start after ~0.3 MB), then
        # the remaining X columns; spread round-robin over four queues
        xt_r = xt_d.rearrange("(c p) (sb s) -> c sb p s", c=DC, sb=QB)
        dma_q = [nc.sync, nc.scalar, nc.gpsimd, nc.scalar]
        qi = [0]

        def dma(out, in_):
            dma_q[qi[0] % 4].dma_start(out=out, in_=in_)
            qi[0] += 1

        xt_c = xt_sb[:].rearrange("p c (sb s) -> p c sb s", sb=QB)
        for c in range(DC):
            for w_sb, w_d in ((wk_sb, wk_d), (wq_sb, wq_d), (wv_sb, wv_d)):
                w_r = w_d.rearrange("(c p) h -> c p h", c=DC)
                dma(w_sb[:, c, :], w_r[c])
            dma(xt_c[:, c, 0, :], xt_r[c, 0])
        for sb in range(1, QB):
            for c in range(DC):
                dma(xt_c[:, c, sb, :], xt_r[c, sb])

        def load_wo_bo():
            for c in range(HC):
                nc.sync.dma_start(
                    out=wo_sb[:, c, :],
                    in_=wo_d.rearrange("(c p) v -> c p v", c=HC)[c],
                )
            bo_row = rb_pool.tile([1, VD], F32, tag="bo_row")
            nc.sync.dma_start(out=bo_row[0:1, :], in_=bo_d[None, :])
            nc.gpsimd.partition_broadcast(bo_sb[:], bo_row[0:1, :])

        # ---- phase A job machinery (QKV projections) ----
        def emit_pa_job(kind, a, b, ps, off, d_lo=0, d_hi=DC):
            for i in range(d_lo, d_hi):
                d = (off + i) % DC
                if kind == "q":
                    lhsT = wq_sb[:, d, a * 128 : (a + 1) * 128]
                    rhs = xt_sb[:, d, b * 512 : (b + 1) * 512]
                elif kind == "k":
                    lhsT = wk_sb[:, d, a * 128 : (a + 1) * 128]
                    rhs = xt_sb[:, d, b * 512 : (b + 1) * 512]
                else:
                    lhsT = xt_sb[:, d, a * 128 : (a + 1) * 128]
                    rhs = wv_sb[:, d, :]
                nc.tensor.matmul(ps, lhsT, rhs, start=(i == 0), stop=(i == DC - 1))
            if d_hi < DC:
                return
            if kind == "q":
                nc.vector.tensor_scalar_add(
                    out=qt_sb[:, a, b * 512 : (b + 1) * 512],
                    in0=ps,
                    scalar1=bq_sb[:, a : a + 1],
                )
            elif kind == "k":
                nc.vector.tensor_copy(
                    out=kt_sb[:, a, b * 512 : (b + 1) * 512], in_=ps
                )
            else:
                nc.vector.tensor_copy(
                    out=v_sb[:, a, :, 0:DH],
                    in_=ps.rearrange("p (h d) -> p h d", h=4),
                )

        pa_count = [0]

        def emit_pa_batch(jobs):
            for j in range(0, len(jobs), 2):
                ps2 = mm_ps_pool.tile([128, 2, 512], F32, tag="mm")
                for s_i, job in enumerate(jobs[j : j + 2]):
                    kind = job[0]
                    ps = ps2[:, s_i, 0:256] if kind == "v" else ps2[:, s_i, :]
                    emit_pa_job(*job, ps, pa_count[0] % DC)
                    pa_count[0] += 1

        def pa_thunk(jobs):
            def thunk():
                with tc.high_priority(offset=-60):
                    emit_pa_batch(jobs)

            return thunk

        def pa_half_thunks(job):
            # one job as two 3-matmul halves sharing a psum tile
            state = {}

            def first():
                with tc.high_priority(offset=-60):
                    inj_ps = mm_ps_pool.tile([128, 2, 512], F32, tag="mm")
                    state["ps"] = inj_ps
                    state["off"] = pa_count[0] % DC
                    pa_count[0] += 1
                    ps = (
                        inj_ps[:, 0, 0:256] if job[0] == "v" else inj_ps[:, 0, :]
                    )
                    state["psv"] = ps
                    emit_pa_job(*job, ps, state["off"], 0, DC // 2)

            def second():
                with tc.high_priority(offset=-60):
                    emit_pa_job(*job, state["psv"], state["off"], DC // 2, DC)

            return first, second

        # ---- phase C job ----
        def emit_y(qt_i):
            y_ps = mm_ps_pool.tile([128, 2, 512], F32, tag="mm")
            for c in range(HC):
                if c < HC_L:
                    lhsT = att_sb[:, c, qt_i * 128 : (qt_i + 1) * 128]
                else:
                    lhsT = att_rsb[:, c - HC_L, qt_i * 128 : (qt_i + 1) * 128]
                nc.tensor.matmul(
                    y_ps[:, 0, :],
                    lhsT,
                    wo_sb[:, c, 0:512],
                    start=(c == 0),
                    stop=(c == HC - 1),
                )
                nc.tensor.matmul(
                    y_ps[:, 1, 0 : VD - 512],
                    lhsT,
                    wo_sb[:, c, 512:VD],
                    start=(c == 0),
                    stop=(c == HC - 1),
                )
            y_sb = y_sb_pool.tile([128, VD], F32, tag="ysb")
            nc.vector.tensor_add(y_sb[:, 0:512], y_ps[:, 0, :], bo_sb[:, 0:512])
            nc.vector.tensor_add(
                y_sb[:, 512:VD], y_ps[:, 1, 0 : VD - 512], bo_sb[:, 512:VD]
            )
            nc.sync.dma_start(
                out=y_d.rearrange("(t p) v -> t p v", p=128)[qt_i], in_=y_sb[:]
            )

        def y_thunk(qt_i):
            def thunk():
                with tc.high_priority(offset=-60):
                    emit_y(qt_i)

            return thunk

        # ---- attended^T pair exchange (gpsimd queue only) ----
        def emit_exchange():
            nc.gpsimd.dma_start(
                out=send_b[:].rearrange("p (c q) -> p c q", c=HC_L),
                in_=att_sb[:, :, SQ:S],
            )
            nc.gpsimd.collective_compute(
                "AllGather",
                mybir.AluOpType.bypass,
                replica_groups=[[0, 1], [2, 3], [4, 5], [6, 7]],
                ins=[send_b.opt()],
                outs=[gath_b.opt()],
            )
            # partner block = (block0 + block1) - my own contribution;
            # the add rides the CCE in the DMA datapath, the subtract is
            # one DVE op. Uniform across cores (no parity branching).
            g = gath_b[:].rearrange("(blk p) (c q) -> blk p c q", blk=2, c=HC_L)
            nc.gpsimd.dma_start(out=att_rsb[:], in_=g[0])
            nc.gpsimd.dma_start(
                out=att_rsb[:], in_=g[1], accum_op=mybir.AluOpType.add
            )
            nc.vector.tensor_sub(
                out=att_rsb[:], in0=att_rsb[:], in1=att_sb[:, :, SQ:S]
            )

        # ---- phase B attention iteration ----
        pend = []
        period = [0]

        def flush_pend(lag=0):
            while pend and pend[0][0] <= period[0] - lag:
                pend.pop(0)[1]()

        def emit_attention(qb, hp, inject=None, lag=3):
            h0, h1 = 2 * hp, 2 * hp + 1
            att0 = att_ps_pool.tile([128, 512], F32, tag="att0")
            att1 = att_ps_pool.tile([128, 512], F32, tag="att1")
            qs = qt_sb[:, hp, qb * 512 : (qb + 1) * 512]

            def attended(kt, e):
                def thunk():
                    nc.tensor.matmul(
                        att0[0 : DH + 1, :],
                        v_sb[:, kt, h0, :],
                        e[:, 0, :],
                        start=(kt == 0),
                        stop=(kt == KT - 1),
                    )
                    nc.tensor.matmul(
                        att1[0 : DH + 1, :],
                        v_sb[:, kt, h1, :],
                        e[:, 1, :],
                        start=(kt == 0),
                        stop=(kt == KT - 1),
                    )

                return thunk

            def epilogue():
                # normalize rows 0:64 by 1/rowsum (row 64); odd head
                # shifts to partitions 64:128 via a small SBUF->SBUF DMA
                for h, att in ((h0, att0), (h1, att1)):
                    atmp = tmp_pool.tile([DH + 1, 512], F32, tag="atmp")
                    nc.vector.tensor_copy(atmp[:], att[0 : DH + 1, :])
                    nc.vector.reciprocal_approx_fast(
                        atmp[DH : DH + 1, :], atmp[DH : DH + 1, :]
                    )
                    rb = rb_pool.tile([DH, 512], F32, tag="rb")
                    nc.sync.dma_start(
                        out=rb[:], in_=_bcast_rows(atmp[DH : DH + 1, :], DH)
                    )
                    dst_cols = att_sb[:, hp, qb * 512 : (qb + 1) * 512]
                    if h % 2 == 0:
                        nc.vector.tensor_mul(dst_cols[0:64, :], atmp[0:DH, :], rb[:])
                    else:
                        tmp_n = tmp_pool.tile([64, 512], BF16, tag="tmp")
                        nc.vector.tensor_mul(tmp_n[:], atmp[0:DH, :], rb[:])
                        nc.sync.dma_start(out=dst_cols[64:128, :], in_=tmp_n[:])

            for kt in range(KT):
                if inject and kt in inject:
                    for t in inject[kt]:
                        t()
                s_ps = mm_ps_pool.tile([128, 2, 512], F32, tag="mm")
                ks = kt_sb[:, hp, kt * 128 : (kt + 1) * 128]
                nc.tensor.matmul(
                    s_ps[:, 0, :], ks[0:64, :], qs[0:64, :], start=True, stop=True
                )
                nc.tensor.matmul(
                    s_ps[:, 1, :], ks[64:128, :], qs[64:128, :], start=True, stop=True
                )
                e = e_pool.tile([128, 2, 512], BF16, tag="e")
                nc.scalar.activation(
                    out=e[:],
                    in_=s_ps[:],
                    func=mybir.ActivationFunctionType.Exp,
                    bias=zero_sb[:, 0:1],
                    scale=0.125,
                )
                flush_pend(lag=lag)
                pend.append((period[0], attended(kt, e)))
                period[0] += 1
            pend.append((period[0] - 1, epilogue))

        # ---- emission schedule ----
        # iteration order: local qb0 first (its X/K columns arrive first),
        # then the remote half (qb2, qb3) so the exchange can fire early,
        # then (0,1) and finally qb1 whose Y jobs form the tail.
        order = [
            (0, 0), (2, 0), (2, 1), (3, 0), (3, 1), (0, 1), (1, 0), (1, 1),
        ]

        emit_pa_batch([("k", 0, 0), ("q", 0, 0)])

        injections = {}

        def add_inj(it, kt, thunk):
            injections.setdefault(it, {}).setdefault(kt, []).append(thunk)

        # it1 (0,0): remaining K-hc0 columns chase the key stream; all 16
        # V jobs (paired) stream one pair per two steps; Q-hc0-qb2 at the
        # end for it2
        add_inj((0, 0), 1, pa_thunk([("k", 0, 1)]))
        add_inj((0, 0), 5, pa_thunk([("k", 0, 2)]))
        add_inj((0, 0), 9, pa_thunk([("k", 0, 3)]))
        for j in range(8):
            add_inj((0, 0), 2 * j, pa_thunk([("v", 2 * j, 0), ("v", 2 * j + 1, 0)]))
        add_inj((0, 0), 13, pa_thunk([("q", 0, 2)]))
        # it2 (2,0): K-hc1 as halves + Q chunks for it3/it4
        kq_halves = []
        for job in [("k", 1, sb) for sb in range(QB)] + [("q", 1, 2)]:
            kq_halves.extend(pa_half_thunks(job))
        for j, th in enumerate(kq_halves):
            add_inj((2, 0), 1 + j, th)
        add_inj((2, 0), 12, pa_thunk([("q", 0, 3)]))
        # it3 (2,1): wo/bo load + Q-hc1-qb3
        add_inj((2, 1), 2, load_wo_bo)
        add_inj((2, 1), 7, pa_thunk([("q", 1, 3)]))
        # it4 (3,0): Q-hc1-qb0 (for it6)
        add_inj((3, 0), 4, pa_thunk([("q", 1, 0)]))
        # it5 (3,1): Q-hc0-qb1 (for it7)
        add_inj((3, 1), 4, pa_thunk([("q", 0, 1)]))
        # it6 (0,1): exchange + Q-hc1-qb1 (for it8). The exchange reads
        # it5's att columns, whose epilogue is flushed at it6 step ~3 --
        # inject at step 4 so emission order matches data order.
        add_inj((0, 1), 4, emit_exchange)
        add_inj((0, 1), 6, pa_thunk([("q", 1, 1)]))
        # it8 (1,1): Y jobs for qb0 rows
        for j, qt_i in enumerate((2, 6, 10, 14)):
            add_inj((1, 1), qt_i, y_thunk(j))

        for it_i, (qb, hp) in enumerate(order):
            lag = 1 if it_i == len(order) - 1 else 3
            emit_attention(qb, hp, injections.get((qb, hp)), lag=lag)
        flush_pend()
        for qt_i in range(4, 8):
            emit_y(qt_i)


_BUILT = None


def _build():
    global _BUILT
    if _BUILT is None:
        nc = bacc.Bacc(
            "TRN2", target_bir_lowering=False, debug=False, num_devices=N_CORES
        )
        with tile.TileContext(nc) as tc:
            _kernel_body(tc)
        nc.compile()
        _BUILT = nc
    return _BUILT


def _prepare_in_maps(text_embeds, Wq, bq, Wk, bk, Wv, bv, Wo, bo):
    import ml_dtypes

    bf16 = ml_dtypes.bfloat16
    text_embeds = np.asarray(text_embeds, np.float32)
    Wq = np.asarray(Wq, np.float32)
    Wk = np.asarray(Wk, np.float32)
    Wv = np.asarray(Wv, np.float32)
    Wo32 = np.asarray(Wo, np.float32)
    bq = np.asarray(bq, np.float32)
    bo2 = (
        np.asarray(bo, np.float64)
        + np.asarray(bv, np.float64) @ Wo32.astype(np.float64)
    ).astype(np.float32)
    in_maps = []
    for core in range(N_CORES):
        b, half = divmod(core, 2)
        xt = text_embeds[b].T  # [D, S]
        if half:
            xt = np.roll(xt, -SQ, axis=1)
        cols = slice(HL * half, HL * half + HL)
        other = slice(HL * (1 - half), HL * (1 - half) + HL)
        wo_perm = np.concatenate([Wo32[cols], Wo32[other]], axis=0)
        in_maps.append(
            {
                "xt": np.ascontiguousarray(xt.astype(bf16)),
                "wq": np.ascontiguousarray(Wq[:, cols].astype(bf16)),
                "wk": np.ascontiguousarray(Wk[:, cols].astype(bf16)),
                "wv": np.ascontiguousarray(Wv[:, cols].astype(bf16)),
                "bq": np.ascontiguousarray(bq[cols]),
                "wo": np.ascontiguousarray(wo_perm.astype(bf16)),
                "bo2": bo2,
            }
        )
    return in_maps


def _assemble(results):
    out = np.empty((B, S, VD), np.float32)
    for core in range(N_CORES):
        b, half = divmod(core, 2)
        out[b, half * SQ : (half + 1) * SQ] = results[core]["y"]
    return out


def run(trace=False, **inputs):
    nc = _build()
    in_maps = _prepare_in_maps(**inputs)
    res = bass_utils.run_bass_kernel_spmd(
        nc, in_maps, core_ids=list(range(N_CORES)), trace=trace
    )
    return _assemble(res.results), res


def kernel(**inputs):
    out, _ = run(trace=False, **inputs)
    return out


# revision 15
# speedup vs baseline: 1.1987x; 1.1987x over previous
"""Multi-head self-attention + projector, Trainium2 Bass kernel, 8 NeuronCores.

Reference computation (per batch b):
    Q = X @ Wq + bq; K = X @ Wk + bk; V = X @ Wv + bv      (X: [S, D])
    per head h: P_h = softmax(Q_h K_h^T / sqrt(dh)); A_h = P_h V_h
    Y = concat_h(A_h) @ Wo + bo

Sharding: core i handles batch i//2; the PAIR (2b, 2b+1) splits the
HEADS (tensor parallel): each core projects Q/K/V for its 4 heads only,
over the full sequence, runs attention for those heads over all 2048
queries, then the pair exchanges attended^T halves (0.5 MB AllGather)
so each core can compute Y for its 1024 output rows with all 8 heads.
The host rolls odd cores' X columns by -1024 and permutes Wq/Wk/Wv
columns + Wo rows so one SPMD program serves all 8 cores; with the
roll, "my output queries" are local columns 0:1024 on every core and
the partner's needed half is local columns 1024:2048 on every core.

Algebraic simplifications (all exact w.r.t. softmax):
  - bk dropped: per-query constant in scores, softmax cancels it.
  - bv folded into the output bias on host (softmax rows sum to 1).
  - no max-subtraction in softmax: scores are O(1) for these inputs.

Device pipeline per core:
  phase A: Q^T[256,2048] (+bq), K^T[256,2048], V[2048,4,65] (bf16 with
           a per-head ones column for free softmax row sums). Only
           K-chunk0/Q-chunk0 for the first query block run up front;
           everything else streams into the attention iterations.
  phase B: 8 iterations (qb in 4 blocks of 512 queries x hp in 2 local
           head pairs): stream keys in 128-chunks: scoresT via PE (head
           pair row-tiled 0:64/64:128, concurrent), exp on ACT (bf16),
           attended^T accumulation on PE. Normalize by the row-sum row
           (reciprocal in place + stride-0 broadcast DMA); odd head
           shifts to partitions 64:128 via a small SBUF->SBUF DMA.
  exchange: after the 5th iteration the remote query half of att^T is
           AllGather'd within the pair (gpsimd queue only); the partner
           block is selected with parity-conditional DMAs.
  phase C: Y[q,768] = [attT_local; attT_remote].T @ Wo + bo' per
           128-row q-tile, interleaved into the last iteration + tail.
"""

import numpy as np

import concourse.bass as bass
import concourse.mybir as mybir
import concourse.tile as tile
from concourse import bacc, bass_utils

F32 = mybir.dt.float32
BF16 = mybir.dt.bfloat16

B, S, D, HID, HEADS, DH, VD = 4, 2048, 768, 512, 8, 64, 768
N_CORES = 8
SQ = S // 2  # output query rows per core
HL = HID // 2  # local hidden width (4 heads)
HC_L = HL // 128  # 2 local hidden chunks
DC = D // 128  # 6 contraction chunks for the projections
HC = HID // 128  # 4 hidden chunks for phase C
KT = S // 128  # 16 key chunks
QB = S // 512  # 4 query blocks of 512
HPL = 2  # local head pairs


def _bcast_rows(row_ap, n):
    # [1, F] AP -> [1, n, F] AP: free-dim stride-0 repeat of one SBUF row
    ap = [list(row_ap.ap[0])] + [[0, n]] + [list(d) for d in row_ap.ap[1:]]
    return bass.AP(row_ap.tensor, row_ap.offset, ap)


def _kernel_body(tc):
    nc = tc.nc
    xt_d = nc.dram_tensor("xt", [D, S], BF16, kind="ExternalInput").ap()
    wq_d = nc.dram_tensor("wq", [D, HL], BF16, kind="ExternalInput").ap()
    wk_d = nc.dram_tensor("wk", [D, HL], BF16, kind="ExternalInput").ap()
    wv_d = nc.dram_tensor("wv", [D, HL], BF16, kind="ExternalInput").ap()
    bq_d = nc.dram_tensor("bq", [HL], F32, kind="ExternalInput").ap()
    wo_d = nc.dram_tensor("wo", [HID, VD], BF16, kind="ExternalInput").ap()
    bo_d = nc.dram_tensor("bo2", [VD], F32, kind="ExternalInput").ap()
    y_d = nc.dram_tensor("y", [SQ, VD], F32, kind="ExternalOutput").ap()

    with (
        tc.tile_pool(name="persist", bufs=1) as persist,
        tc.tile_pool(name="mm_ps", bufs=3, space="PSUM") as mm_ps_pool,
        tc.tile_pool(name="att_ps", bufs=1, space="PSUM") as att_ps_pool,
        tc.tile_pool(name="pa_sbuf", bufs=1) as pa_sbuf,
        tc.tile_pool(name="e_pool", bufs=12) as e_pool,
        tc.tile_pool(name="rb_pool", bufs=4) as rb_pool,
        tc.tile_pool(name="tmp_pool", bufs=4) as tmp_pool,
        tc.tile_pool(name="y_sb", bufs=2) as y_sb_pool,
        tc.tile_pool(name="dram", bufs=1, space="DRAM") as dram_pool,
    ):
        # ---- persistent SBUF tensors ----
        wo_sb = persist.tile([128, HC, VD], BF16)
        bo_sb = persist.tile([128, VD], F32)
        bq_sb = persist.tile([128, HC_L], F32)
        qt_sb = persist.tile([128, HC_L, S], BF16)
        kt_sb = persist.tile([128, HC_L, S], BF16)
        # V in [seq, head, 65]: per local head [V(64) | ones]
        v_sb = persist.tile([128, KT, 4, DH + 1], BF16)
        att_sb = persist.tile([128, HC_L, S], BF16)
        att_rsb = persist.tile([128, HC_L, SQ], BF16)
        zero_sb = persist.tile([128, 1], F32)

        nc.vector.memset(zero_sb[:], 0.0)
        nc.vector.memset(v_sb[:, :, :, DH : DH + 1], 1.0)
        nc.sync.dma_start(out=bq_sb[:], in_=bq_d.rearrange("(c p) -> p c", c=HC_L))

        xt_sb = pa_sbuf.tile([128, DC, S], BF16)
        wq_sb = pa_sbuf.tile([128, DC, HL], BF16)
        wk_sb = pa_sbuf.tile([128, DC, HL], BF16)
        wv_sb = pa_sbuf.tile([128, DC, HL], BF16)

        send_b = dram_pool.tile([128, HC_L * SQ], BF16)
        gath_b = dram_pool.tile([2 * 128, HC_L * SQ], BF16)

        # input DMAs: weight chunk c + the first query-block column of X
        # first (so the upfront K/Q jobs can start after ~0.3 MB), then
        # the remaining X columns; spread round-robin over four queues
        xt_r = xt_d.rearrange("(c p) (sb s) -> c sb p s", c=DC, sb=QB)
        dma_q = [nc.sync, nc.scalar, nc.gpsimd, nc.scalar]
        qi = [0]

        def dma(out, in_):
            dma_q[qi[0] % 4].dma_start(out=out, in_=in_)
            qi[0] += 1

        xt_c = xt_sb[:].rearrange("p c (sb s) -> p c sb s", sb=QB)
        for c in range(DC):
            for w_sb, w_d in ((wk_sb, wk_d), (wq_sb, wq_d), (wv_sb, wv_d)):
                w_r = w_d.rearrange("(c p) h -> c p h", c=DC)
                dma(w_sb[:, c, :], w_r[c])
            dma(xt_c[:, c, 0, :], xt_r[c, 0])
        for sb in range(1, QB):
            for c in range(DC):
                dma(xt_c[:, c, sb, :], xt_r[c, sb])

        def load_wo_bo():
            for c in range(HC):
                nc.sync.dma_start(
                    out=wo_sb[:, c, :],
                    in_=wo_d.rearrange("(c p) v -> c p v", c=HC)[c],
                )
            bo_row = rb_pool.tile([1, VD], F32, tag="bo_row")
            nc.sync.dma_start(out=bo_row[0:1, :], in_=bo_d[None, :])
            nc.gpsimd.partition_broadcast(bo_sb[:], bo_row[0:1, :])

        # ---- phase A job machinery (QKV projections) ----
        def emit_pa_job(kind, a, b, ps, off, d_lo=0, d_hi=DC):
            for i in range(d_lo, d_hi):
                d = (off + i) % DC
                if kind == "q":
                    lhsT = wq_sb[:, d, a * 128 : (a + 1) * 128]
                    rhs = xt_sb[:, d, b * 512 : (b + 1) * 512]
                elif kind == "k":
                    lhsT = wk_sb[:, d, a * 128 : (a + 1) * 128]
                    rhs = xt_sb[:, d, b * 512 : (b + 1) * 512]
                else:
                    lhsT = xt_sb[:, d, a * 128 : (a + 1) * 128]
                    rhs = wv_sb[:, d, :]
                nc.tensor.matmul(ps, lhsT, rhs, start=(i == 0), stop=(i == DC - 1))
            if d_hi < DC:
                return
            if kind == "q":
                nc.vector.tensor_scalar_add(
                    out=qt_sb[:, a, b * 512 : (b + 1) * 512],
                    in0=ps,
                    scalar1=bq_sb[:, a : a + 1],
                )
            elif kind == "k":
                nc.vector.tensor_copy(
                    out=kt_sb[:, a, b * 512 : (b + 1) * 512], in_=ps
                )
            else:
                nc.vector.tensor_copy(
                    out=v_sb[:, a, :, 0:DH],
                    in_=ps.rearrange("p (h d) -> p h d", h=4),
                )

        pa_count = [0]

        def emit_pa_batch(jobs):
            for j in range(0, len(jobs), 2):
                ps2 = mm_ps_pool.tile([128, 2, 512], F32, tag="mm")
                for s_i, job in enumerate(jobs[j : j + 2]):
                    kind = job[0]
                    ps = ps2[:, s_i, 0:256] if kind == "v" else ps2[:, s_i, :]
                    emit_pa_job(*job, ps, pa_count[0] % DC)
                    pa_count[0] += 1

        def pa_thunk(jobs):
            def thunk():
                with tc.high_priority(offset=-60):
                    emit_pa_batch(jobs)

            return thunk

        def pa_half_thunks(job):
            # one job as two 3-matmul halves sharing a psum tile
            state = {}

            def first():
                with tc.high_priority(offset=-60):
                    inj_ps = mm_ps_pool.tile([128, 2, 512], F32, tag="mm")
                    state["ps"] = inj_ps
                    state["off"] = pa_count[0] % DC
                    pa_count[0] += 1
                    ps = (
                        inj_ps[:, 0, 0:256] if job[0] == "v" else inj_ps[:, 0, :]
                    )
                    state["psv"] = ps
                    emit_pa_job(*job, ps, state["off"], 0, DC // 2)

            def second():
                with tc.high_priority(offset=-60):
                    emit_pa_job(*job, state["psv"], state["off"], DC // 2, DC)

            return first, second

        # ---- phase C job ----
        def emit_y(qt_i):
            y_ps = mm_ps_pool.tile([128, 2, 512], F32, tag="mm")
            for c in range(HC):
                if c < HC_L:
                    lhsT = att_sb[:, c, qt_i * 128 : (qt_i + 1) * 128]
                else:
                    lhsT = att_rsb[:, c - HC_L, qt_i * 128 : (qt_i + 1) * 128]
                nc.tensor.matmul(
                    y_ps[:, 0, :],
                    lhsT,
                    wo_sb[:, c, 0:512],
                    start=(c == 0),
                    stop=(c == HC - 1),
                )
                nc.tensor.matmul(
                    y_ps[:, 1, 0 : VD - 512],
                    lhsT,
                    wo_sb[:, c, 512:VD],
                    start=(c == 0),
                    stop=(c == HC - 1),
                )
            y_sb = y_sb_pool.tile([128, VD], F32, tag="ysb")
            nc.vector.tensor_add(y_sb[:, 0:512], y_ps[:, 0, :], bo_sb[:, 0:512])
            nc.vector.tensor_add(
                y_sb[:, 512:VD], y_ps[:, 1, 0 : VD - 512], bo_sb[:, 512:VD]
            )
            nc.sync.dma_start(
                out=y_d.rearrange("(t p) v -> t p v", p=128)[qt_i], in_=y_sb[:]
            )

        def y_thunk(qt_i):
            def thunk():
                with tc.high_priority(offset=-60):
                    emit_y(qt_i)

            return thunk

        # ---- attended^T pair exchange ----
        # send + collective ride the sync queue (its pending epilogue
        # DMAs can absorb the completion wait); the gathered-block reads
        # ride gpsimd a bit later so its epilogue broadcasts stay fluid.
        def emit_exchange_send():
            nc.gpsimd.dma_start(
                out=send_b[:].rearrange("p (c q) -> p c q", c=HC_L),
                in_=att_sb[:, :, SQ:S],
            )
            nc.gpsimd.collective_compute(
                "AllGather",
                mybir.AluOpType.bypass,
                replica_groups=[[0, 1], [2, 3], [4, 5], [6, 7]],
                ins=[send_b.opt()],
                outs=[gath_b.opt()],
            )

        def emit_exchange_recv():
            # partner block = (block0 + block1) - my own contribution;
            # the add rides the CCE in the DMA datapath, the subtract is
            # one DVE op. Uniform across cores (no parity branching).
            g = gath_b[:].rearrange("(blk p) (c q) -> blk p c q", blk=2, c=HC_L)
            nc.gpsimd.dma_start(out=att_rsb[:], in_=g[0])
            nc.gpsimd.dma_start(
                out=att_rsb[:], in_=g[1], accum_op=mybir.AluOpType.add
            )
            nc.vector.tensor_sub(
                out=att_rsb[:], in0=att_rsb[:], in1=att_sb[:, :, SQ:S]
            )

        # ---- phase B attention iteration ----
        pend = []
        period = [0]

        def flush_pend(lag=0):
            while pend and pend[0][0] <= period[0] - lag:
                pend.pop(0)[1]()

        def emit_attention(qb, hp, inject=None, lag=3):
            h0, h1 = 2 * hp, 2 * hp + 1
            att0 = att_ps_pool.tile([128, 512], F32, tag="att0")
            att1 = att_ps_pool.tile([128, 512], F32, tag="att1")
            qs = qt_sb[:, hp, qb * 512 : (qb + 1) * 512]

            def attended(kt, e):
                def thunk():
                    nc.tensor.matmul(
                        att0[0 : DH + 1, :],
                        v_sb[:, kt, h0, :],
                        e[:, 0, :],
                        start=(kt == 0),
                        stop=(kt == KT - 1),
                    )
                    nc.tensor.matmul(
                        att1[0 : DH + 1, :],
                        v_sb[:, kt, h1, :],
                        e[:, 1, :],
                        start=(kt == 0),
                        stop=(kt == KT - 1),
                    )

                return thunk

            def epilogue():
                # normalize rows 0:64 by 1/rowsum (row 64); odd head
                # shifts to partitions 64:128 via a small SBUF->SBUF DMA
                for h, att in ((h0, att0), (h1, att1)):
                    atmp = tmp_pool.tile([DH + 1, 512], F32, tag="atmp")
                    nc.vector.tensor_copy(atmp[:], att[0 : DH + 1, :])
                    rec0 = rb_pool.tile([1, 512], F32, tag="rec0")
                    nc.sync.dma_start(rec0[0:1, :], atmp[DH : DH + 1, :])
                    nc.vector.reciprocal_approx_fast(rec0[0:1, :], rec0[0:1, :])
                    rb = rb_pool.tile([DH, 512], F32, tag="rb")
                    nc.gpsimd.partition_broadcast(rb[:], rec0[0:1, :])
                    dst_cols = att_sb[:, hp, qb * 512 : (qb + 1) * 512]
                    if h % 2 == 0:
                        nc.vector.tensor_mul(dst_cols[0:64, :], atmp[0:DH, :], rb[:])
                    else:
                        tmp_n = tmp_pool.tile([64, 512], BF16, tag="tmp")
                        nc.vector.tensor_mul(tmp_n[:], atmp[0:DH, :], rb[:])
                        nc.sync.dma_start(out=dst_cols[64:128, :], in_=tmp_n[:])

            for kt in range(KT):
                if inject and kt in inject:
                    for t in inject[kt]:
                        t()
                s_ps = mm_ps_pool.tile([128, 2, 512], F32, tag="mm")
                ks = kt_sb[:, hp, kt * 128 : (kt + 1) * 128]
                nc.tensor.matmul(
                    s_ps[:, 0, :], ks[0:64, :], qs[0:64, :], start=True, stop=True
                )
                nc.tensor.matmul(
                    s_ps[:, 1, :], ks[64:128, :], qs[64:128, :], start=True, stop=True
                )
                e = e_pool.tile([128, 2, 512], BF16, tag="e")
                nc.scalar.activation(
                    out=e[:],
                    in_=s_ps[:],
                    func=mybir.ActivationFunctionType.Exp,
                    bias=zero_sb[:, 0:1],
                    scale=0.125,
                )
                flush_pend(lag=lag)
                pend.append((period[0], attended(kt, e)))
                period[0] += 1
            pend.append((period[0] - 1, epilogue))

        # ---- emission schedule ----
        # iteration order: local qb0 first (its X/K columns arrive first),
        # then the remote half (qb2, qb3) so the exchange can fire early,
        # then (0,1) and finally qb1 whose Y jobs form the tail.
        order = [
            (0, 0), (2, 0), (2, 1), (3, 0), (3, 1), (0, 1), (1, 0), (1, 1),
        ]

        emit_pa_batch([("k", 0, 0), ("q", 0, 0)])

        injections = {}

        def add_inj(it, kt, thunk):
            injections.setdefault(it, {}).setdefault(kt, []).append(thunk)

        # it1 (0,0): remaining K-hc0 columns chase the key stream; all 16
        # V jobs (paired) stream one pair per two steps; Q-hc0-qb2 at the
        # end for it2
        add_inj((0, 0), 1, pa_thunk([("k", 0, 1)]))
        add_inj((0, 0), 5, pa_thunk([("k", 0, 2)]))
        add_inj((0, 0), 9, pa_thunk([("k", 0, 3)]))
        for j in range(8):
            add_inj((0, 0), 2 * j, pa_thunk([("v", 2 * j, 0), ("v", 2 * j + 1, 0)]))
        add_inj((0, 0), 13, pa_thunk([("q", 0, 2)]))
        # it2 (2,0): K-hc1 as halves + Q chunks for it3/it4
        kq_halves = []
        for job in [("k", 1, sb) for sb in range(QB)] + [("q", 1, 2)]:
            kq_halves.extend(pa_half_thunks(job))
        for j, th in enumerate(kq_halves):
            add_inj((2, 0), 1 + j, th)
        add_inj((2, 0), 12, pa_thunk([("q", 0, 3)]))
        # it3 (2,1): wo/bo load + Q-hc1-qb3
        add_inj((2, 1), 2, load_wo_bo)
        add_inj((2, 1), 7, pa_thunk([("q", 1, 3)]))
        # it4 (3,0): Q-hc1-qb0 (for it6)
        add_inj((3, 0), 4, pa_thunk([("q", 1, 0)]))
        # it5 (3,1): Q-hc0-qb1 (for it7)
        add_inj((3, 1), 4, pa_thunk([("q", 0, 1)]))
        # it6 (0,1): exchange send + Q-hc1-qb1 (for it8). The send reads
        # it5's att columns, whose epilogue is flushed at it6 step ~3 --
        # inject at step 4 so emission order matches data order. The
        # receive side lands in it7 once the collective has completed.
        add_inj((0, 1), 4, emit_exchange_send)
        add_inj((0, 1), 6, pa_thunk([("q", 1, 1)]))
        add_inj((1, 0), 4, emit_exchange_recv)
        # it8 (1,1): Y jobs for qb0 rows
        for j, qt_i in enumerate((2, 6, 10, 14)):
            add_inj((1, 1), qt_i, y_thunk(j))

        for it_i, (qb, hp) in enumerate(order):
            lag = 1 if it_i == len(order) - 1 else 3
            emit_attention(qb, hp, injections.get((qb, hp)), lag=lag)
        flush_pend()
        for qt_i in range(4, 8):
            emit_y(qt_i)


_BUILT = None


def _build():
    global _BUILT
    if _BUILT is None:
        nc = bacc.Bacc(
            "TRN2", target_bir_lowering=False, debug=False, num_devices=N_CORES
        )
        with tile.TileContext(nc) as tc:
            _kernel_body(tc)
        nc.compile()
        _BUILT = nc
    return _BUILT


def _prepare_in_maps(text_embeds, Wq, bq, Wk, bk, Wv, bv, Wo, bo):
    import ml_dtypes

    bf16 = ml_dtypes.bfloat16
    text_embeds = np.asarray(text_embeds, np.float32)
    Wq = np.asarray(Wq, np.float32)
    Wk = np.asarray(Wk, np.float32)
    Wv = np.asarray(Wv, np.float32)
    Wo32 = np.asarray(Wo, np.float32)
    bq = np.asarray(bq, np.float32)
    bo2 = (
        np.asarray(bo, np.float64)
        + np.asarray(bv, np.float64) @ Wo32.astype(np.float64)
    ).astype(np.float32)
    in_maps = []
    for core in range(N_CORES):
        b, half = divmod(core, 2)
        xt = text_embeds[b].T  # [D, S]
        if half:
            xt = np.roll(xt, -SQ, axis=1)
        cols = slice(HL * half, HL * half + HL)
        other = slice(HL * (1 - half), HL * (1 - half) + HL)
        wo_perm = np.concatenate([Wo32[cols], Wo32[other]], axis=0)
        in_maps.append(
            {
                "xt": np.ascontiguousarray(xt.astype(bf16)),
                "wq": np.ascontiguousarray(Wq[:, cols].astype(bf16)),
                "wk": np.ascontiguousarray(Wk[:, cols].astype(bf16)),
                "wv": np.ascontiguousarray(Wv[:, cols].astype(bf16)),
                "bq": np.ascontiguousarray(bq[cols]),
                "wo": np.ascontiguousarray(wo_perm.astype(bf16)),
                "bo2": bo2,
            }
        )
    return in_maps


def _assemble(results):
    out = np.empty((B, S, VD), np.float32)
    for core in range(N_CORES):
        b, half = divmod(core, 2)
        out[b, half * SQ : (half + 1) * SQ] = results[core]["y"]
    return out


def run(trace=False, **inputs):
    nc = _build()
    in_maps = _prepare_in_maps(**inputs)
    res = bass_utils.run_bass_kernel_spmd(
        nc, in_maps, core_ids=list(range(N_CORES)), trace=trace
    )
    return _assemble(res.results), res


def kernel(**inputs):
    out, _ = run(trace=False, **inputs)
    return out


# revision 17
# speedup vs baseline: 1.2060x; 1.0060x over previous
"""Multi-head self-attention + projector, Trainium2 Bass kernel, 8 NeuronCores.

Reference computation (per batch b):
    Q = X @ Wq + bq; K = X @ Wk + bk; V = X @ Wv + bv      (X: [S, D])
    per head h: P_h = softmax(Q_h K_h^T / sqrt(dh)); A_h = P_h V_h
    Y = concat_h(A_h) @ Wo + bo

Sharding: core i handles batch i//2; the PAIR (2b, 2b+1) splits the
HEADS (tensor parallel): each core projects Q/K/V for its 4 heads only,
over the full sequence, runs attention for those heads over all 2048
queries, then the pair exchanges attended^T halves (0.5 MB AllGather)
so each core can compute Y for its 1024 output rows with all 8 heads.
The host rolls odd cores' X columns by -1024 and permutes Wq/Wk/Wv
columns + Wo rows so one SPMD program serves all 8 cores; with the
roll, "my output queries" are local columns 0:1024 on every core and
the partner's needed half is local columns 1024:2048 on every core.

Algebraic simplifications (all exact w.r.t. softmax):
  - bk dropped: per-query constant in scores, softmax cancels it.
  - bv folded into the output bias on host (softmax rows sum to 1).
  - no max-subtraction in softmax: scores are O(1) for these inputs.

Device pipeline per core:
  phase A: Q^T[256,2048] (+bq), K^T[256,2048], V[2048,4,65] (bf16 with
           a per-head ones column for free softmax row sums). Only
           K-chunk0/Q-chunk0 for the first query block run up front;
           everything else streams into the attention iterations.
  phase B: 8 iterations (qb in 4 blocks of 512 queries x hp in 2 local
           head pairs): stream keys in 128-chunks: scoresT via PE (head
           pair row-tiled 0:64/64:128, concurrent), exp on ACT (bf16),
           attended^T accumulation on PE. Normalize by the row-sum row
           (reciprocal in place + stride-0 broadcast DMA); odd head
           shifts to partitions 64:128 via a small SBUF->SBUF DMA.
  exchange: after the 5th iteration the remote query half of att^T is
           AllGather'd within the pair (gpsimd queue only); the partner
           block is selected with parity-conditional DMAs.
  phase C: Y[q,768] = [attT_local; attT_remote].T @ Wo + bo' per
           128-row q-tile, interleaved into the last iteration + tail.
"""

import numpy as np

import concourse.bass as bass
import concourse.mybir as mybir
import concourse.tile as tile
from concourse import bacc, bass_utils

F32 = mybir.dt.float32
BF16 = mybir.dt.bfloat16

B, S, D, HID, HEADS, DH, VD = 4, 2048, 768, 512, 8, 64, 768
N_CORES = 8
SQ = S // 2  # output query rows per core
HL = HID // 2  # local hidden width (4 heads)
HC_L = HL // 128  # 2 local hidden chunks
DC = D // 128  # 6 contraction chunks for the projections
HC = HID // 128  # 4 hidden chunks for phase C
KT = S // 128  # 16 key chunks
QB = S // 512  # 4 query blocks of 512
HPL = 2  # local head pairs


def _bcast_rows(row_ap, n):
    # [1, F] AP -> [1, n, F] AP: free-dim stride-0 repeat of one SBUF row
    ap = [list(row_ap.ap[0])] + [[0, n]] + [list(d) for d in row_ap.ap[1:]]
    return bass.AP(row_ap.tensor, row_ap.offset, ap)


def _kernel_body(tc):
    nc = tc.nc
    xt_d = nc.dram_tensor("xt", [D, S], BF16, kind="ExternalInput").ap()
    wq_d = nc.dram_tensor("wq", [D, HL], BF16, kind="ExternalInput").ap()
    wk_d = nc.dram_tensor("wk", [D, HL], BF16, kind="ExternalInput").ap()
    wv_d = nc.dram_tensor("wv", [D, HL], BF16, kind="ExternalInput").ap()
    bq_d = nc.dram_tensor("bq", [HL], F32, kind="ExternalInput").ap()
    wo_d = nc.dram_tensor("wo", [HID, VD], BF16, kind="ExternalInput").ap()
    bo_d = nc.dram_tensor("bo2", [VD], F32, kind="ExternalInput").ap()
    y_d = nc.dram_tensor("y", [SQ, VD], F32, kind="ExternalOutput").ap()

    with (
        tc.tile_pool(name="persist", bufs=1) as persist,
        tc.tile_pool(name="mm_ps", bufs=3, space="PSUM") as mm_ps_pool,
        tc.tile_pool(name="att_ps", bufs=1, space="PSUM") as att_ps_pool,
        tc.tile_pool(name="pa_sbuf", bufs=1) as pa_sbuf,
        tc.tile_pool(name="e_pool", bufs=12) as e_pool,
        tc.tile_pool(name="rb_pool", bufs=4) as rb_pool,
        tc.tile_pool(name="tmp_pool", bufs=4) as tmp_pool,
        tc.tile_pool(name="y_sb", bufs=2) as y_sb_pool,
        tc.tile_pool(name="dram", bufs=1, space="DRAM") as dram_pool,
    ):
        # ---- persistent SBUF tensors ----
        wo_sb = persist.tile([128, HC, VD], BF16)
        bo_sb = persist.tile([128, VD], F32)
        bq_sb = persist.tile([128, HC_L], F32)
        qt_sb = persist.tile([128, HC_L, S], BF16)
        kt_sb = persist.tile([128, HC_L, S], BF16)
        # V in [seq, head, 65]: per local head [V(64) | ones]
        v_sb = persist.tile([128, KT, 4, DH + 1], BF16)
        att_sb = persist.tile([128, HC_L, S], BF16)
        att_rsb = persist.tile([128, HC_L, SQ], BF16)
        zero_sb = persist.tile([128, 1], F32)

        nc.vector.memset(zero_sb[:], 0.0)
        nc.vector.memset(v_sb[:, :, :, DH : DH + 1], 1.0)
        nc.sync.dma_start(out=bq_sb[:], in_=bq_d.rearrange("(c p) -> p c", c=HC_L))

        xt_sb = pa_sbuf.tile([128, DC, S], BF16)
        wq_sb = pa_sbuf.tile([128, DC, HL], BF16)
        wk_sb = pa_sbuf.tile([128, DC, HL], BF16)
        wv_sb = pa_sbuf.tile([128, DC, HL], BF16)

        send_b = dram_pool.tile([128, HC_L * SQ], BF16)
        gath_b = dram_pool.tile([2 * 128, HC_L * SQ], BF16)

        # input DMAs: weight chunk c + the first query-block column of X
        # first (so the upfront K/Q jobs can start after ~0.3 MB), then
        # the remaining X columns; spread round-robin over four queues
        xt_r = xt_d.rearrange("(c p) (sb s) -> c sb p s", c=DC, sb=QB)
        dma_q = [nc.sync, nc.scalar, nc.gpsimd, nc.scalar]
        qi = [0]

        def dma(out, in_):
            dma_q[qi[0] % 4].dma_start(out=out, in_=in_)
            qi[0] += 1

        xt_c = xt_sb[:].rearrange("p c (sb s) -> p c sb s", sb=QB)
        wk_r = wk_d.rearrange("(c p) h -> c p h", c=DC)
        wq_r = wq_d.rearrange("(c p) h -> c p h", c=DC)
        wv_r = wv_d.rearrange("(c p) h -> c p h", c=DC)
        # wave A: feeds the upfront K (sb0) and Q (sb2) jobs
        for c in range(DC):
            dma(wk_sb[:, c, :], wk_r[c])
            dma(wq_sb[:, c, :], wq_r[c])
            dma(xt_c[:, c, 0, :], xt_r[c, 0])
            dma(xt_c[:, c, 2, :], xt_r[c, 2])
        # wave B: feeds it1's V jobs and the chasing K columns
        for c in range(DC):
            dma(wv_sb[:, c, :], wv_r[c])
            dma(xt_c[:, c, 1, :], xt_r[c, 1])
        for c in range(DC):
            dma(xt_c[:, c, 3, :], xt_r[c, 3])

        def load_wo_bo():
            for c in range(HC):
                nc.sync.dma_start(
                    out=wo_sb[:, c, :],
                    in_=wo_d.rearrange("(c p) v -> c p v", c=HC)[c],
                )
            bo_row = rb_pool.tile([1, VD], F32, tag="bo_row")
            nc.sync.dma_start(out=bo_row[0:1, :], in_=bo_d[None, :])
            nc.gpsimd.partition_broadcast(bo_sb[:], bo_row[0:1, :])

        # ---- phase A job machinery (QKV projections) ----
        def emit_pa_job(kind, a, b, ps, off, d_lo=0, d_hi=DC):
            for i in range(d_lo, d_hi):
                d = (off + i) % DC
                if kind == "q":
                    lhsT = wq_sb[:, d, a * 128 : (a + 1) * 128]
                    rhs = xt_sb[:, d, b * 512 : (b + 1) * 512]
                elif kind == "k":
                    lhsT = wk_sb[:, d, a * 128 : (a + 1) * 128]
                    rhs = xt_sb[:, d, b * 512 : (b + 1) * 512]
                else:
                    lhsT = xt_sb[:, d, a * 128 : (a + 1) * 128]
                    rhs = wv_sb[:, d, :]
                nc.tensor.matmul(ps, lhsT, rhs, start=(i == 0), stop=(i == DC - 1))
            if d_hi < DC:
                return
            if kind == "q":
                nc.vector.tensor_scalar_add(
                    out=qt_sb[:, a, b * 512 : (b + 1) * 512],
                    in0=ps,
                    scalar1=bq_sb[:, a : a + 1],
                )
            elif kind == "k":
                nc.vector.tensor_copy(
                    out=kt_sb[:, a, b * 512 : (b + 1) * 512], in_=ps
                )
            else:
                nc.vector.tensor_copy(
                    out=v_sb[:, a, :, 0:DH],
                    in_=ps.rearrange("p (h d) -> p h d", h=4),
                )

        pa_count = [0]

        def emit_pa_batch(jobs):
            for j in range(0, len(jobs), 2):
                ps2 = mm_ps_pool.tile([128, 2, 512], F32, tag="mm")
                for s_i, job in enumerate(jobs[j : j + 2]):
                    kind = job[0]
                    ps = ps2[:, s_i, 0:256] if kind == "v" else ps2[:, s_i, :]
                    emit_pa_job(*job, ps, pa_count[0] % DC)
                    pa_count[0] += 1

        def pa_thunk(jobs):
            def thunk():
                with tc.high_priority(offset=-60):
                    emit_pa_batch(jobs)

            return thunk

        def pa_half_thunks(job):
            # one job as two 3-matmul halves sharing a psum tile
            state = {}

            def first():
                with tc.high_priority(offset=-60):
                    inj_ps = mm_ps_pool.tile([128, 2, 512], F32, tag="mm")
                    state["ps"] = inj_ps
                    state["off"] = pa_count[0] % DC
                    pa_count[0] += 1
                    ps = (
                        inj_ps[:, 0, 0:256] if job[0] == "v" else inj_ps[:, 0, :]
                    )
                    state["psv"] = ps
                    emit_pa_job(*job, ps, state["off"], 0, DC // 2)

            def second():
                with tc.high_priority(offset=-60):
                    emit_pa_job(*job, state["psv"], state["off"], DC // 2, DC)

            return first, second

        # ---- phase C job ----
        def emit_y(qt_i):
            y_ps = mm_ps_pool.tile([128, 2, 512], F32, tag="mm")
            for c in range(HC):
                if c < HC_L:
                    lhsT = att_sb[:, c, qt_i * 128 : (qt_i + 1) * 128]
                else:
                    lhsT = att_rsb[:, c - HC_L, qt_i * 128 : (qt_i + 1) * 128]
                nc.tensor.matmul(
                    y_ps[:, 0, :],
                    lhsT,
                    wo_sb[:, c, 0:512],
                    start=(c == 0),
                    stop=(c == HC - 1),
                )
                nc.tensor.matmul(
                    y_ps[:, 1, 0 : VD - 512],
                    lhsT,
                    wo_sb[:, c, 512:VD],
                    start=(c == 0),
                    stop=(c == HC - 1),
                )
            y_sb = y_sb_pool.tile([128, VD], F32, tag="ysb")
            nc.vector.tensor_add(y_sb[:, 0:512], y_ps[:, 0, :], bo_sb[:, 0:512])
            nc.vector.tensor_add(
                y_sb[:, 512:VD], y_ps[:, 1, 0 : VD - 512], bo_sb[:, 512:VD]
            )
            nc.sync.dma_start(
                out=y_d.rearrange("(t p) v -> t p v", p=128)[qt_i], in_=y_sb[:]
            )

        def y_thunk(qt_i):
            def thunk():
                with tc.high_priority(offset=-60):
                    emit_y(qt_i)

            return thunk

        # ---- attended^T pair exchange ----
        # send + collective ride the sync queue (its pending epilogue
        # DMAs can absorb the completion wait); the gathered-block reads
        # ride gpsimd a bit later so its epilogue broadcasts stay fluid.
        def emit_exchange_send():
            nc.gpsimd.dma_start(
                out=send_b[:].rearrange("p (c q) -> p c q", c=HC_L),
                in_=att_sb[:, :, SQ:S],
            )
            nc.gpsimd.collective_compute(
                "AllGather",
                mybir.AluOpType.bypass,
                replica_groups=[[0, 1], [2, 3], [4, 5], [6, 7]],
                ins=[send_b.opt()],
                outs=[gath_b.opt()],
            )

        def emit_exchange_recv():
            # partner block = (block0 + block1) - my own contribution;
            # the add rides the CCE in the DMA datapath, the subtract is
            # one DVE op. Uniform across cores (no parity branching).
            g = gath_b[:].rearrange("(blk p) (c q) -> blk p c q", blk=2, c=HC_L)
            nc.gpsimd.dma_start(out=att_rsb[:], in_=g[0])
            nc.gpsimd.dma_start(
                out=att_rsb[:], in_=g[1], accum_op=mybir.AluOpType.add
            )
            nc.vector.tensor_sub(
                out=att_rsb[:], in0=att_rsb[:], in1=att_sb[:, :, SQ:S]
            )

        # ---- phase B attention iteration ----
        pend = []
        period = [0]

        def flush_pend(lag=0):
            while pend and pend[0][0] <= period[0] - lag:
                pend.pop(0)[1]()

        def emit_attention(qb, hp, inject=None, lag=3):
            h0, h1 = 2 * hp, 2 * hp + 1
            att0 = att_ps_pool.tile([128, 512], F32, tag="att0")
            att1 = att_ps_pool.tile([128, 512], F32, tag="att1")
            qs = qt_sb[:, hp, qb * 512 : (qb + 1) * 512]

            def attended(kt, e):
                def thunk():
                    nc.tensor.matmul(
                        att0[0 : DH + 1, :],
                        v_sb[:, kt, h0, :],
                        e[:, 0, :],
                        start=(kt == 0),
                        stop=(kt == KT - 1),
                    )
                    nc.tensor.matmul(
                        att1[0 : DH + 1, :],
                        v_sb[:, kt, h1, :],
                        e[:, 1, :],
                        start=(kt == 0),
                        stop=(kt == KT - 1),
                    )

                return thunk

            def epilogue():
                # normalize rows 0:64 by 1/rowsum (row 64); odd head
                # shifts to partitions 64:128 via a small SBUF->SBUF DMA
                for h, att in ((h0, att0), (h1, att1)):
                    atmp = tmp_pool.tile([DH + 1, 512], F32, tag="atmp")
                    nc.vector.tensor_copy(atmp[:], att[0 : DH + 1, :])
                    rec0 = rb_pool.tile([1, 512], F32, tag="rec0")
                    nc.sync.dma_start(rec0[0:1, :], atmp[DH : DH + 1, :])
                    nc.vector.reciprocal_approx_fast(rec0[0:1, :], rec0[0:1, :])
                    rb = rb_pool.tile([DH, 512], F32, tag="rb")
                    nc.gpsimd.partition_broadcast(rb[:], rec0[0:1, :])
                    dst_cols = att_sb[:, hp, qb * 512 : (qb + 1) * 512]
                    if h % 2 == 0:
                        nc.vector.tensor_mul(dst_cols[0:64, :], atmp[0:DH, :], rb[:])
                    else:
                        tmp_n = tmp_pool.tile([64, 512], BF16, tag="tmp")
                        nc.vector.tensor_mul(tmp_n[:], atmp[0:DH, :], rb[:])
                        nc.sync.dma_start(out=dst_cols[64:128, :], in_=tmp_n[:])

            for kt in range(KT):
                if inject and kt in inject:
                    for t in inject[kt]:
                        t()
                s_ps = mm_ps_pool.tile([128, 2, 512], F32, tag="mm")
                ks = kt_sb[:, hp, kt * 128 : (kt + 1) * 128]
                nc.tensor.matmul(
                    s_ps[:, 0, :], ks[0:64, :], qs[0:64, :], start=True, stop=True
                )
                nc.tensor.matmul(
                    s_ps[:, 1, :], ks[64:128, :], qs[64:128, :], start=True, stop=True
                )
                e = e_pool.tile([128, 2, 512], BF16, tag="e")
                nc.scalar.activation(
                    out=e[:],
                    in_=s_ps[:],
                    func=mybir.ActivationFunctionType.Exp,
                    bias=zero_sb[:, 0:1],
                    scale=0.125,
                )
                flush_pend(lag=lag)
                pend.append((period[0], attended(kt, e)))
                period[0] += 1
            pend.append((period[0] - 1, epilogue))

        # ---- emission schedule ----
        # iteration order: the remote query half (qb2, qb3) first so the
        # exchange can fire after it4 with ~4 iterations of slack, then
        # the local half; qb1 last, its Y jobs forming the tail.
        order = [
            (2, 0), (3, 0), (2, 1), (3, 1), (0, 0), (0, 1), (1, 0), (1, 1),
        ]

        emit_pa_batch([("k", 0, 0), ("q", 0, 2)])

        injections = {}

        def add_inj(it, kt, thunk):
            injections.setdefault(it, {}).setdefault(kt, []).append(thunk)

        # it1 (2,0): remaining K-hc0 columns chase the key stream; all 16
        # V jobs (paired) stream one pair per two steps; Q-hc0-qb3 at the
        # end for it2
        add_inj((2, 0), 1, pa_thunk([("k", 0, 1)]))
        add_inj((2, 0), 5, pa_thunk([("k", 0, 2)]))
        add_inj((2, 0), 9, pa_thunk([("k", 0, 3)]))
        for j in range(8):
            add_inj((2, 0), 2 * j, pa_thunk([("v", 2 * j, 0), ("v", 2 * j + 1, 0)]))
        add_inj((2, 0), 13, pa_thunk([("q", 0, 3)]))
        # it2 (3,0): K-hc1 as halves + Q-hc1-qb2 for it3
        kq_halves = []
        for job in [("k", 1, sb) for sb in range(QB)] + [("q", 1, 2)]:
            kq_halves.extend(pa_half_thunks(job))
        for j, th in enumerate(kq_halves):
            add_inj((3, 0), 1 + j, th)
        # it3 (2,1): wo/bo load + Q-hc1-qb3 (for it4)
        add_inj((2, 1), 2, load_wo_bo)
        add_inj((2, 1), 7, pa_thunk([("q", 1, 3)]))
        # it4 (3,1): Q-hc0-qb0 (for it5)
        add_inj((3, 1), 4, pa_thunk([("q", 0, 0)]))
        # it5 (0,0): exchange send + Q-hc1-qb0 (for it6). The send reads
        # it4's att columns, whose epilogue is flushed at it5 step ~3 --
        # inject at step 4 so emission order matches data order. The
        # receive side lands in it6 once the collective has completed.
        add_inj((0, 0), 4, emit_exchange_send)
        add_inj((0, 0), 8, pa_thunk([("q", 1, 0)]))
        # it6 (0,1): exchange receive + Q-hc0-qb1 (for it7)
        add_inj((0, 1), 4, emit_exchange_recv)
        add_inj((0, 1), 8, pa_thunk([("q", 0, 1)]))
        # it7 (1,0): Q-hc1-qb1 (for it8) + Y jobs for qb0 rows (qb0's
        # last epilogue flushes at it7 step ~3, so Y starts at step 4)
        add_inj((1, 0), 2, pa_thunk([("q", 1, 1)]))
        for j, qt_i in enumerate((4, 8, 11, 14)):
            add_inj((1, 0), qt_i, y_thunk(j))

        for it_i, (qb, hp) in enumerate(order):
            lag = 1 if it_i == len(order) - 1 else 3
            emit_attention(qb, hp, injections.get((qb, hp)), lag=lag)
        flush_pend()
        for qt_i in range(4, 8):
            emit_y(qt_i)


_BUILT = None


def _build():
    global _BUILT
    if _BUILT is None:
        nc = bacc.Bacc(
            "TRN2", target_bir_lowering=False, debug=False, num_devices=N_CORES
        )
        with tile.TileContext(nc) as tc:
            _kernel_body(tc)
        nc.compile()
        _BUILT = nc
    return _BUILT


def _prepare_in_maps(text_embeds, Wq, bq, Wk, bk, Wv, bv, Wo, bo):
    import ml_dtypes

    bf16 = ml_dtypes.bfloat16
    text_embeds = np.asarray(text_embeds, np.float32)
    Wq = np.asarray(Wq, np.float32)
    Wk = np.asarray(Wk, np.float32)
    Wv = np.asarray(Wv, np.float32)
    Wo32 = np.asarray(Wo, np.float32)
    bq = np.asarray(bq, np.float32)
    bo2 = (
        np.asarray(bo, np.float64)
        + np.asarray(bv, np.float64) @ Wo32.astype(np.float64)
    ).astype(np.float32)
    in_maps = []
    for core in range(N_CORES):
        b, half = divmod(core, 2)
        xt = text_embeds[b].T  # [D, S]
        if half:
            xt = np.roll(xt, -SQ, axis=1)
        cols = slice(HL * half, HL * half + HL)
        other = slice(HL * (1 - half), HL * (1 - half) + HL)
        wo_perm = np.concatenate([Wo32[cols], Wo32[other]], axis=0)
        in_maps.append(
            {
                "xt": np.ascontiguousarray(xt.astype(bf16)),
                "wq": np.ascontiguousarray(Wq[:, cols].astype(bf16)),
                "wk": np.ascontiguousarray(Wk[:, cols].astype(bf16)),
                "wv": np.ascontiguousarray(Wv[:, cols].astype(bf16)),
                "bq": np.ascontiguousarray(bq[cols]),
                "wo": np.ascontiguousarray(wo_perm.astype(bf16)),
                "bo2": bo2,
            }
        )
    return in_maps


def _assemble(results):
    out = np.empty((B, S, VD), np.float32)
    for core in range(N_CORES):
        b, half = divmod(core, 2)
        out[b, half * SQ : (half + 1) * SQ] = results[core]["y"]
    return out


def run(trace=False, **inputs):
    nc = _build()
    in_maps = _prepare_in_maps(**inputs)
    res = bass_utils.run_bass_kernel_spmd(
        nc, in_maps, core_ids=list(range(N_CORES)), trace=trace
    )
    return _assemble(res.results), res


def kernel(**inputs):
    out, _ = run(trace=False, **inputs)
    return out
